# revision 36
# baseline (speedup 1.0000x reference)
"""Trainium2 Bass kernel for nn_BertForRelationExtractionV2.

Strategy (pure data parallel, per sharding hint):
  - Shard batch (B=32) across 8 cores, 4 samples/core; replicate weights.
  - Samples are sorted by entity-span size and dealt round-robin to cores, so
    SPMD slot s on every core has a per-slot query-chunk budget (ragged
    sequence handling that is identical across cores). Only queries inside
    the (dilated) entity spans contribute to the output, so the query side of
    attention is gathered to those positions on the host.
  - Host-side prep: layout transposes, normalized span masks, pos-emb gather,
    and algebraic folds:
      * cross-attention softmax over a single key == 1, so cross collapses to
        a linear map; composed with the O-projection onto the span mean c2.
      * O-projection is applied after the masked span-mean (linearity).
      * LayerNorm+classifier folded: logits = rs*(x@A.T - mu*s1) + c0,
        A = cls_w * ln_g, s1 = A.sum(1), c0 = cls_w@ln_b + cls_b.
  - Device per sample: K/V projections over the full sequence, Q projection
    over gathered span queries, per-head scores (no max-sub; inputs bounded so
    exp is safe), exp with free-dim accum for softmax sums, masked-mean folded
    into k-space weights w[k] = sum_q exp[q,k]*m[q]/sum[q], c = v.T @ w.
  - Matmul operands are float16 (full PE rate, fp32 PSUM accumulation).
"""

import math
import sys
from contextlib import ExitStack

import numpy as np

for _p in ("/opt/trn_rl_repo",):
    if _p not in sys.path:
        sys.path.insert(0, _p)

import concourse.bass as bass
import concourse.mybir as mybir
import concourse.tile as tile
from concourse import bacc
from concourse.bass_utils import run_bass_kernel_spmd

F32 = mybir.dt.float32
F16 = mybir.dt.float16
AF = mybir.ActivationFunctionType
ALU = mybir.AluOpType

B, S, H = 32, 512, 768
NH, HD = 8, 96            # self-attention heads / head dim
NCORE = 8
BPC = B // NCORE          # samples per core
KC = H // 128             # contraction chunks over H
QC = S // 128             # chunks over full sequence
NL = 14                   # labels
SIX_H = 6 * H             # 4608 concat features
JC = SIX_H // 128         # 36 feature chunks
LN_EPS = 1e-5
SCALE = 1.0 / math.sqrt(HD)


def _r32(x):
    return np.ascontiguousarray(x, dtype=np.float32)


def _r16(x):
    return np.ascontiguousarray(np.asarray(x, np.float32), dtype=np.float16)


def _prep_shared(inp):
    """Replicated (weight) arrays in device layouts."""
    def wT(W):  # torch Linear weight (out,in) -> lhsT layout [128, KC, H]
        return _r32(np.asarray(W, np.float32).T.reshape(KC, 128, H).transpose(1, 0, 2))

    def wT_headpad(W):  # lhsT with heads padded to 128 cols: [128, KC, NH*128]
        t = np.asarray(W, np.float32).T.reshape(KC, 128, NH, HD)  # [kc, p, h, d]
        out = np.zeros((128, KC, NH * 128), np.float32)
        out.reshape(128, KC, NH, 128)[:, :, :, :HD] = t.transpose(1, 0, 2, 3)
        return out

    def wpad(M):  # (out,in) -> head-padded lhsT layout [128, NH, H], rows 96..127 zero
        out = np.zeros((128, NH, H), np.float32)
        t = np.asarray(M, np.float32).T  # (in=d, out=o)
        out[:HD] = t.reshape(NH, HD, H).transpose(1, 0, 2)
        return _r32(out)

    def colchunk(v):  # (768,) -> [128, KC]
        return _r32(np.asarray(v, np.float32).reshape(KC, 128).T)

    def headchunk(v):  # (768,) -> [HD, NH]
        return _r32(np.asarray(v, np.float32).reshape(NH, HD).T)

    cin_w = np.asarray(inp["cross_in_w"], np.float32)
    cin_b = np.asarray(inp["cross_in_b"], np.float32)
    cout_w = np.asarray(inp["cross_out_w"], np.float32)
    cout_b = np.asarray(inp["cross_out_b"], np.float32)
    vw = cin_w[2 * H:]
    vb = cin_b[2 * H:]
    Wo_ = np.asarray(inp["Wo"], np.float32)
    bo_ = np.asarray(inp["bo"], np.float32)
    # cross = e2 @ (cross_out_w @ vw).T + (cross_out_w @ vb + cross_out_b), and
    # e2 = c2 @ Wo.T + bo, so compose both linear maps onto c2 directly.
    Wcv = cout_w @ vw
    Wco = Wcv @ Wo_
    bco = Wcv @ bo_ + cout_w @ vb + cout_b

    cls_w = np.asarray(inp["cls_w"], np.float32)
    ln_g = np.asarray(inp["ln_g"], np.float32)
    ln_b = np.asarray(inp["ln_b"], np.float32)
    cls_b = np.asarray(inp["cls_b"], np.float32)
    A = cls_w * ln_g[None, :]                       # (NL, 6H)
    atx = np.ones((128, JC, NL + 1), np.float32)    # col NL stays 1.0 (sum column)
    atx[:, :, :NL] = A.T.reshape(JC, 128, NL).transpose(1, 0, 2)
    s1 = A.sum(axis=1)
    c0 = cls_w @ ln_b + cls_b

    return {
        "wqT": _r16(wT_headpad(inp["Wq"])), "wkT": _r16(wT_headpad(inp["Wk"])),
        "wvT": _r16(wT(inp["Wv"])),
        "woT": _r16(wpad(inp["Wo"])), "wcoT": _r16(wpad(Wco)),
        "bq": headchunk(inp["bq"]), "bk": headchunk(inp["bk"]),
        "bv": _r32(np.broadcast_to(np.asarray(inp["bv"], np.float32)[None, :], (128, H))),
        "bo": colchunk(inp["bo"]), "bco": colchunk(bco),
        "atx": _r16(atx),
        "s1r": _r32(np.broadcast_to(s1[None, :], (BPC, NL))),
        "c0r": _r32(np.broadcast_to(c0[None, :], (BPC, NL))),
    }


def _spans(ep):
    """Per-sample union of dilated entity spans -> (positions, m1, m2)."""
    pos = np.arange(S)
    s1 = max(int(ep[0]) - 2, 0)
    e1 = min(int(ep[1]) + 2, S)
    s2 = max(int(ep[2]) - 2, 0)
    e2 = min(int(ep[3]) + 2, S)
    m1 = ((pos >= s1) & (pos < e1)).astype(np.float32)
    m2 = ((pos >= s2) & (pos < e2)).astype(np.float32)
    m1 /= m1.sum()
    m2 /= m2.sum()
    union = np.nonzero((m1 > 0) | (m2 > 0))[0]
    return union, m1, m2


def _prep_percore(inp):
    """Sample permutation, per-slot budgets, and per-core sharded arrays."""
    seq = np.asarray(inp["seq_out"], np.float32)          # (B, S, H)
    ep = np.asarray(inp["entity_positions"]).astype(np.int64)

    # xT [B, 128, KC, S] fp16 (K/V projections consume the full sequence)
    xT = seq.transpose(0, 2, 1).reshape(B, KC, 128, S).transpose(0, 2, 1, 3)
    xT = xT.astype(np.float16)

    spans = [_spans(ep[b]) for b in range(B)]
    chunks = np.array([max(1, (len(u) + 127) // 128) for u, _, _ in spans])

    # sort samples by chunk count desc; rank r -> core r%8, slot r//8
    order = np.argsort(-chunks, kind="stable")
    budgets = tuple(int(chunks[order[s * NCORE]]) for s in range(BPC))

    pe = np.asarray(inp["pos_table"], np.float32)[ep].sum(axis=1)  # (B, H)

    cores = []
    for c in range(NCORE):
        sids = [int(order[s * NCORE + c]) for s in range(BPC)]
        m = {"xT": np.ascontiguousarray(xT[sids])}
        for s, b_id in enumerate(sids):
            union, m1, m2 = spans[b_id]
            L = budgets[s] * 128
            idx = np.zeros(L, np.int64)
            idx[:len(union)] = union
            xq = seq[b_id].T[:, idx]                     # (H, L) gathered queries
            xq = xq.reshape(KC, 128, L).transpose(1, 0, 2)
            m[f"xq{s}"] = _r16(xq)
            mk = np.zeros((L, 2), np.float32)
            mk[:len(union), 0] = m1[union]
            mk[:len(union), 1] = m2[union]
            m[f"mk{s}"] = _r32(mk.reshape(budgets[s], 128, 2).transpose(1, 0, 2))
        peT = pe[sids].T.reshape(KC, 128, BPC).transpose(1, 0, 2)
        m["posT"] = _r16(peT)
        cores.append(m)
    perm = [int(order[s * NCORE + c]) for c in range(NCORE) for s in range(BPC)]
    return cores, budgets, perm


def _emit(tc, nc, d, out_ap, budgets):
    ctx = ExitStack()
    with ctx:
        wp = ctx.enter_context(tc.tile_pool(name="weights", bufs=1))
        xp = ctx.enter_context(tc.tile_pool(name="x", bufs=2))
        vp = ctx.enter_context(tc.tile_pool(name="v", bufs=2))
        qkp = ctx.enter_context(tc.tile_pool(name="qk", bufs=2))
        epool = ctx.enter_context(tc.tile_pool(name="exp", bufs=8))
        sp = ctx.enter_context(tc.tile_pool(name="small", bufs=8))
        tp = ctx.enter_context(tc.tile_pool(name="tail", bufs=1))
        ps_big = ctx.enter_context(tc.tile_pool(name="psA", bufs=2, space="PSUM"))
        ps_sc = ctx.enter_context(tc.tile_pool(name="psS", bufs=3, space="PSUM"))
        ps_sm = ctx.enter_context(tc.tile_pool(name="psW", bufs=3, space="PSUM"))

        def load(name, shape, dt=F32):
            t = wp.tile(list(shape), dt, tag=name)
            nc.sync.dma_start(t[:], d[name])
            return t

        # DMA emission in first-use order: sample-0 activations and the QKV
        # weights first, tail-only weights after the sample loop so they
        # stream in while attention computes.
        xts, xqs, mks = {}, {}, {}

        def prefetch(s):
            if s >= BPC or s in xts:
                return
            xT_ = xp.tile([128, KC, S], F16, tag="xT", name=f"xt{s}")
            nc.sync.dma_start(xT_[:], d["xT"][s])
            xts[s] = xT_
            xqs[s] = load(f"xq{s}", (128, KC, budgets[s] * 128), F16)
            mks[s] = load(f"mk{s}", (128, budgets[s], 2))

        # sample-0 critical loads split in half so the first V-projection
        # accumulation group starts as soon as the leading chunks land
        xT0 = xp.tile([128, KC, S], F16, tag="xT", name="xt0")
        wv = wp.tile([128, KC, H], F16, tag="wvT")
        nc.sync.dma_start(xT0[:, 0:3, :], d["xT"][0][:, 0:3, :])
        nc.sync.dma_start(wv[:, 0:3, :], d["wvT"][:, 0:3, :])
        nc.sync.dma_start(xT0[:, 3:KC, :], d["xT"][0][:, 3:KC, :])
        nc.sync.dma_start(wv[:, 3:KC, :], d["wvT"][:, 3:KC, :])
        xts[0] = xT0
        xqs[0] = load("xq0", (128, KC, budgets[0] * 128), F16)
        mks[0] = load("mk0", (128, budgets[0], 2))
        wq = load("wqT", (128, KC, NH * 128), F16)
        wk = load("wkT", (128, KC, NH * 128), F16)
        bq = load("bq", (HD, NH))
        bk = load("bk", (HD, NH))
        bv = load("bv", (128, H))

        ones = wp.tile([128, 1], F16, tag="ones")
        nc.vector.memset(ones[:], 1.0)

        c_pad = tp.tile([128, NH, 2 * BPC], F16, tag="c_pad")
        nc.vector.memset(c_pad[:], 0.0)
        combT = tp.tile([128, JC, BPC], F16, tag="combT")

        def mm(ps_ap, lhsT, rhs, start, stop):
            nc.tensor.matmul(ps_ap, lhsT=lhsT, rhs=rhs, start=start, stop=stop)

        for smp in range(BPC):
            prefetch(smp + 1)  # next sample's loads stream under this compute
            QB = budgets[smp]
            LQ = QB * 128
            xT = xts[smp]

            # V projection: v[s, o] with sequence position on partitions
            v_sb = vp.tile([128, QC, H], F16, tag="v")
            for sc in range(QC):
                for half in range(2):
                    pv_full = ps_big.tile([128, 512], F32, tag="a", name="pv")
                    pv = pv_full[:, :384]
                    for kc in range(KC):
                        mm(pv[:], xT[:, kc, sc * 128:(sc + 1) * 128],
                           wv[:, kc, half * 384:(half + 1) * 384], kc == 0, kc == KC - 1)
                    nc.vector.tensor_tensor(v_sb[:, sc, half * 384:(half + 1) * 384],
                                            pv[:], bv[:, half * 384:(half + 1) * 384], ALU.add)

            # Q over gathered span queries; K over the full sequence
            xq = xqs[smp]
            q_sb = qkp.tile([HD, NH, LQ], F16, tag="q")
            k_sb = qkp.tile([HD, NH, S], F16, tag="k")
            for h in range(NH):
                pq = ps_big.tile([128, S], F32, tag="a", name="pq")
                for kc in range(KC):
                    mm(pq[:, :LQ], wq[:, kc, h * 128:(h + 1) * 128], xq[:, kc, :],
                       kc == 0, kc == KC - 1)
                nc.vector.tensor_scalar_add(q_sb[:, h, :], pq[:HD, :LQ], bq[:, h:h + 1])
                pk = ps_big.tile([128, S], F32, tag="a", name="pk")
                for kc in range(KC):
                    mm(pk[:], wk[:, kc, h * 128:(h + 1) * 128], xT[:, kc, :],
                       kc == 0, kc == KC - 1)
                nc.vector.tensor_scalar_add(k_sb[:, h, :], pk[:HD, :], bk[:, h:h + 1])

            # attention, phase-split across heads so the scheduler can pack PE:
            # phase 1: all scores+exp+meff; phase 2: all pw/pc
            exps, meffs = [], []
            for h in range(NH):
                exp_sb = epool.tile([128, QB, S], F16, tag="exp")
                sums = sp.tile([128, QB, 1], F32, tag="sums")
                for qc in range(QB):
                    ps = ps_sc.tile([128, S], F32, tag="ps")
                    mm(ps[:], q_sb[:, h, qc * 128:(qc + 1) * 128], k_sb[:, h, :],
                       True, True)
                    nc.scalar.activation(exp_sb[:, qc, :], ps[:], AF.Exp, scale=SCALE,
                                         accum_out=sums[:, qc, :])
                rec = sp.tile([128, QB, 1], F32, tag="rec")
                nc.vector.reciprocal(rec[:], sums[:])
                meff = sp.tile([128, QB, 2], F16, tag="meff")
                for qc in range(QB):
                    nc.vector.tensor_scalar_mul(meff[:, qc, :], mks[smp][:, qc, :],
                                                rec[:, qc, :])
                exps.append(exp_sb)
                meffs.append(meff)
            for h in range(NH):
                exp_sb, meff = exps[h], meffs[h]
                w_sb = sp.tile([128, QC, 2], F16, tag="wsb")
                for ks in range(QC):
                    pw = ps_sm.tile([128, 2], F32, tag="sm")
                    for qc in range(QB):
                        mm(pw[:], exp_sb[:, qc, ks * 128:(ks + 1) * 128], meff[:, qc, :],
                           qc == 0, qc == QB - 1)
                    nc.vector.tensor_copy(w_sb[:, ks, :], pw[:])
                pc = ps_sm.tile([HD, 2], F32, tag="sm")
                for kc in range(QC):
                    mm(pc[:], v_sb[:, kc, h * HD:(h + 1) * HD], w_sb[:, kc, :],
                       kc == 0, kc == QC - 1)
                nc.vector.tensor_copy(c_pad[:HD, h, smp:smp + 1], pc[:, 0:1])
                nc.vector.tensor_copy(c_pad[:HD, h, BPC + smp:BPC + smp + 1], pc[:, 1:2])

        # tail-only constants (DMAs overlap with the attention phase above)
        wo = load("woT", (128, NH, H), F16)
        wco = load("wcoT", (128, NH, H), F16)
        atx = load("atx", (128, JC, NL + 1), F16)
        bo = load("bo", (128, KC))
        bco = load("bco", (128, KC))
        s1r = load("s1r", (BPC, NL))
        c0r = load("c0r", (BPC, NL))
        nc.sync.dma_start(combT[:, 4 * KC:5 * KC, :], d["posT"])

        # tail: e = c @ Wo.T + bo (and cross via folded Wco), concat blocks
        for c in range(KC):
            pe_ = ps_sm.tile([128, 2 * BPC], F32, tag="sm")
            for h in range(NH):
                mm(pe_[:], wo[:, h, c * 128:(c + 1) * 128], c_pad[:, h, :], h == 0, h == NH - 1)
            nc.scalar.activation(combT[:, c, :], pe_[:, 0:BPC], AF.Identity, bias=bo[:, c:c + 1])
            nc.scalar.activation(combT[:, KC + c, :], pe_[:, BPC:2 * BPC], AF.Identity,
                                 bias=bo[:, c:c + 1])
            px = ps_sm.tile([128, BPC], F32, tag="sm")
            for h in range(NH):
                mm(px[:], wco[:, h, c * 128:(c + 1) * 128], c_pad[:, h, BPC:2 * BPC],
                   h == 0, h == NH - 1)
            nc.scalar.activation(combT[:, 5 * KC + c, :], px[:], AF.Identity, bias=bco[:, c:c + 1])
            nc.vector.tensor_tensor(combT[:, 2 * KC + c, :], combT[:, c, :],
                                    combT[:, KC + c, :], ALU.mult)
            nc.vector.tensor_tensor(combT[:, 3 * KC + c, :], combT[:, c, :],
                                    combT[:, KC + c, :], ALU.subtract)

        # folded LayerNorm + classifier
        sq = tp.tile([128, JC, BPC], F16, tag="sq")
        nc.scalar.square(sq[:], combT[:])
        pxa = ps_sm.tile([BPC, NL + 1], F32, tag="sm")
        psq = ps_sm.tile([BPC, 1], F32, tag="sm")
        for j in range(JC):
            mm(pxa[:], combT[:, j, :], atx[:, j, :], j == 0, j == JC - 1)
            mm(psq[:], sq[:, j, :], ones[:], j == 0, j == JC - 1)
        mu = tp.tile([BPC, 1], F32, tag="mu")
        nc.vector.tensor_scalar_mul(mu[:], pxa[:, NL:NL + 1], 1.0 / SIX_H)
        msq = tp.tile([BPC, 1], F32, tag="msq")
        nc.vector.tensor_scalar_mul(msq[:], psq[:], 1.0 / SIX_H)
        mu2 = tp.tile([BPC, 1], F32, tag="mu2")
        nc.scalar.square(mu2[:], mu[:])
        var = tp.tile([BPC, 1], F32, tag="var")
        nc.vector.tensor_tensor(var[:], msq[:], mu2[:], ALU.subtract)
        epsb = tp.tile([BPC, 1], F32, tag="epsb")
        nc.vector.memset(epsb[:], LN_EPS)
        sd = tp.tile([BPC, 1], F32, tag="sd")
        nc.scalar.activation(sd[:], var[:], AF.Sqrt, bias=epsb[:])
        rs = tp.tile([BPC, 1], F32, tag="rs")
        nc.vector.reciprocal(rs[:], sd[:])
        t1 = tp.tile([BPC, NL], F32, tag="t1")
        nc.vector.tensor_scalar_mul(t1[:], s1r[:], mu[:])
        t2 = tp.tile([BPC, NL], F32, tag="t2")
        nc.vector.tensor_tensor(t2[:], pxa[:, 0:NL], t1[:], ALU.subtract)
        t3 = tp.tile([BPC, NL], F32, tag="t3")
        nc.vector.tensor_scalar_mul(t3[:], t2[:], rs[:])
        lg = tp.tile([BPC, NL], F32, tag="lg")
        nc.vector.tensor_tensor(lg[:], t3[:], c0r[:], ALU.add)
        nc.sync.dma_start(out_ap, lg[:])


_CACHED = {}


def _build(budgets):
    if budgets in _CACHED:
        return _CACHED[budgets]
    nc = bacc.Bacc(trn_type="TRN2", debug=False, num_devices=NCORE)
    d = {}

    def din(name, shape, dt=F32):
        d[name] = nc.dram_tensor(name, list(shape), dt, kind="ExternalInput").ap()

    din("xT", (BPC, 128, KC, S), F16)
    din("wqT", (128, KC, NH * 128), F16)
    din("wkT", (128, KC, NH * 128), F16)
    din("wvT", (128, KC, H), F16)
    din("woT", (128, NH, H), F16)
    din("wcoT", (128, NH, H), F16)
    din("bq", (HD, NH))
    din("bk", (HD, NH))
    din("bv", (128, H))
    din("bo", (128, KC))
    din("bco", (128, KC))
    din("atx", (128, JC, NL + 1), F16)
    din("s1r", (BPC, NL))
    din("c0r", (BPC, NL))
    for s in range(BPC):
        din(f"mk{s}", (128, budgets[s], 2))
        din(f"xq{s}", (128, KC, budgets[s] * 128), F16)
    din("posT", (128, KC, BPC), F16)
    out_ap = nc.dram_tensor("out", [BPC, NL], F32, kind="ExternalOutput").ap()

    with tile.TileContext(nc) as tc:
        _emit(tc, nc, d, out_ap, budgets)
    nc.compile()
    _CACHED[budgets] = nc
    return nc


def run(inputs, trace=False):
    shared = _prep_shared(inputs)
    cores, budgets, perm = _prep_percore(inputs)
    nc = _build(budgets)
    in_maps = [dict(shared, **cores[c]) for c in range(NCORE)]
    res = run_bass_kernel_spmd(nc, in_maps, core_ids=list(range(NCORE)), trace=trace)
    stacked = np.concatenate([res.results[c]["out"] for c in range(NCORE)], axis=0)
    out = np.empty_like(stacked)
    out[perm] = stacked
    return np.ascontiguousarray(out, dtype=np.float32), res


def kernel(**inputs):
    out, _ = run(inputs, trace=False)
    return out


# revision 37
# speedup vs baseline: 1.0170x; 1.0170x over previous
"""Trainium2 Bass kernel for nn_BertForRelationExtractionV2.

Strategy (pure data parallel, per sharding hint):
  - Shard batch (B=32) across 8 cores, 4 samples/core; replicate weights.
  - Samples are sorted by entity-span size and dealt round-robin to cores, so
    SPMD slot s on every core has a per-slot query-chunk budget (ragged
    sequence handling that is identical across cores). Only queries inside
    the (dilated) entity spans contribute to the output, so the query side of
    attention is gathered to those positions on the host.
  - Host-side prep: layout transposes, normalized span masks, pos-emb gather,
    and algebraic folds:
      * cross-attention softmax over a single key == 1, so cross collapses to
        a linear map; composed with the O-projection onto the span mean c2.
      * O-projection is applied after the masked span-mean (linearity).
      * LayerNorm+classifier folded: logits = rs*(x@A.T - mu*s1) + c0,
        A = cls_w * ln_g, s1 = A.sum(1), c0 = cls_w@ln_b + cls_b.
  - Device per sample: K/V projections over the full sequence, Q projection
    over gathered span queries, per-head scores (no max-sub; inputs bounded so
    exp is safe), exp with free-dim accum for softmax sums, masked-mean folded
    into k-space weights w[k] = sum_q exp[q,k]*m[q]/sum[q], c = v.T @ w.
  - Matmul operands are float16 (full PE rate, fp32 PSUM accumulation).
"""

import math
import sys
from contextlib import ExitStack

import numpy as np

for _p in ("/opt/trn_rl_repo",):
    if _p not in sys.path:
        sys.path.insert(0, _p)

import concourse.bass as bass
import concourse.mybir as mybir
import concourse.tile as tile
from concourse import bacc
from concourse.bass_utils import run_bass_kernel_spmd

F32 = mybir.dt.float32
F16 = mybir.dt.float16
AF = mybir.ActivationFunctionType
ALU = mybir.AluOpType

B, S, H = 32, 512, 768
NH, HD = 8, 96            # self-attention heads / head dim
NCORE = 8
BPC = B // NCORE          # samples per core
KC = H // 128             # contraction chunks over H
QC = S // 128             # chunks over full sequence
NL = 14                   # labels
SIX_H = 6 * H             # 4608 concat features
JC = SIX_H // 128         # 36 feature chunks
LN_EPS = 1e-5
SCALE = 1.0 / math.sqrt(HD)


def _r32(x):
    return np.ascontiguousarray(x, dtype=np.float32)


def _r16(x):
    return np.ascontiguousarray(np.asarray(x, np.float32), dtype=np.float16)


def _prep_shared(inp):
    """Replicated (weight) arrays in device layouts."""
    def wT(W):  # torch Linear weight (out,in) -> lhsT layout [128, KC, H]
        return _r32(np.asarray(W, np.float32).T.reshape(KC, 128, H).transpose(1, 0, 2))

    def wT_headpad(W):  # lhsT with heads padded to 128 cols: [128, KC, NH*128]
        t = np.asarray(W, np.float32).T.reshape(KC, 128, NH, HD)  # [kc, p, h, d]
        out = np.zeros((128, KC, NH * 128), np.float32)
        out.reshape(128, KC, NH, 128)[:, :, :, :HD] = t.transpose(1, 0, 2, 3)
        return out

    def wpad(M):  # (out,in) -> head-padded lhsT layout [128, NH, H], rows 96..127 zero
        out = np.zeros((128, NH, H), np.float32)
        t = np.asarray(M, np.float32).T  # (in=d, out=o)
        out[:HD] = t.reshape(NH, HD, H).transpose(1, 0, 2)
        return _r32(out)

    def colchunk(v):  # (768,) -> [128, KC]
        return _r32(np.asarray(v, np.float32).reshape(KC, 128).T)

    def headchunk(v):  # (768,) -> [HD, NH]
        return _r32(np.asarray(v, np.float32).reshape(NH, HD).T)

    cin_w = np.asarray(inp["cross_in_w"], np.float32)
    cin_b = np.asarray(inp["cross_in_b"], np.float32)
    cout_w = np.asarray(inp["cross_out_w"], np.float32)
    cout_b = np.asarray(inp["cross_out_b"], np.float32)
    vw = cin_w[2 * H:]
    vb = cin_b[2 * H:]
    Wo_ = np.asarray(inp["Wo"], np.float32)
    bo_ = np.asarray(inp["bo"], np.float32)
    # cross = e2 @ (cross_out_w @ vw).T + (cross_out_w @ vb + cross_out_b), and
    # e2 = c2 @ Wo.T + bo, so compose both linear maps onto c2 directly.
    Wcv = cout_w @ vw
    Wco = Wcv @ Wo_
    bco = Wcv @ bo_ + cout_w @ vb + cout_b

    cls_w = np.asarray(inp["cls_w"], np.float32)
    ln_g = np.asarray(inp["ln_g"], np.float32)
    ln_b = np.asarray(inp["ln_b"], np.float32)
    cls_b = np.asarray(inp["cls_b"], np.float32)
    A = cls_w * ln_g[None, :]                       # (NL, 6H)
    atx = np.ones((128, JC, NL + 1), np.float32)    # col NL stays 1.0 (sum column)
    atx[:, :, :NL] = A.T.reshape(JC, 128, NL).transpose(1, 0, 2)
    s1 = A.sum(axis=1)
    c0 = cls_w @ ln_b + cls_b

    return {
        "wqT": _r16(wT_headpad(inp["Wq"])), "wkT": _r16(wT_headpad(inp["Wk"])),
        "wvT": _r16(wT(inp["Wv"])),
        "woT": _r16(wpad(inp["Wo"])), "wcoT": _r16(wpad(Wco)),
        "bq": headchunk(inp["bq"]), "bk": headchunk(inp["bk"]),
        "bv": _r32(np.broadcast_to(np.asarray(inp["bv"], np.float32)[None, :], (128, H))),
        "bo": colchunk(inp["bo"]), "bco": colchunk(bco),
        "atx": _r16(atx),
        "s1r": _r32(np.broadcast_to(s1[None, :], (BPC, NL))),
        "c0r": _r32(np.broadcast_to(c0[None, :], (BPC, NL))),
    }


def _spans(ep):
    """Per-sample union of dilated entity spans -> (positions, m1, m2)."""
    pos = np.arange(S)
    s1 = max(int(ep[0]) - 2, 0)
    e1 = min(int(ep[1]) + 2, S)
    s2 = max(int(ep[2]) - 2, 0)
    e2 = min(int(ep[3]) + 2, S)
    m1 = ((pos >= s1) & (pos < e1)).astype(np.float32)
    m2 = ((pos >= s2) & (pos < e2)).astype(np.float32)
    m1 /= m1.sum()
    m2 /= m2.sum()
    union = np.nonzero((m1 > 0) | (m2 > 0))[0]
    return union, m1, m2


def _prep_percore(inp):
    """Sample permutation, per-slot budgets, and per-core sharded arrays."""
    seq = np.asarray(inp["seq_out"], np.float32)          # (B, S, H)
    ep = np.asarray(inp["entity_positions"]).astype(np.int64)

    # xT [B, 128, KC, S] fp16 (K/V projections consume the full sequence)
    xT = seq.transpose(0, 2, 1).reshape(B, KC, 128, S).transpose(0, 2, 1, 3)
    xT = xT.astype(np.float16)

    spans = [_spans(ep[b]) for b in range(B)]
    chunks = np.array([max(1, (len(u) + 127) // 128) for u, _, _ in spans])

    # sort samples by chunk count desc; rank r -> core r%8, slot r//8
    order = np.argsort(-chunks, kind="stable")
    budgets = tuple(int(chunks[order[s * NCORE]]) for s in range(BPC))

    pe = np.asarray(inp["pos_table"], np.float32)[ep].sum(axis=1)  # (B, H)

    cores = []
    for c in range(NCORE):
        sids = [int(order[s * NCORE + c]) for s in range(BPC)]
        m = {"xT": np.ascontiguousarray(xT[sids])}
        for s, b_id in enumerate(sids):
            union, m1, m2 = spans[b_id]
            L = budgets[s] * 128
            idx = np.zeros(L, np.int64)
            idx[:len(union)] = union
            xq = seq[b_id].T[:, idx]                     # (H, L) gathered queries
            xq = xq.reshape(KC, 128, L).transpose(1, 0, 2)
            m[f"xq{s}"] = _r16(xq)
            mk = np.zeros((L, 2), np.float32)
            mk[:len(union), 0] = m1[union]
            mk[:len(union), 1] = m2[union]
            m[f"mk{s}"] = _r32(mk.reshape(budgets[s], 128, 2).transpose(1, 0, 2))
        peT = pe[sids].T.reshape(KC, 128, BPC).transpose(1, 0, 2)
        m["posT"] = _r16(peT)
        cores.append(m)
    perm = [int(order[s * NCORE + c]) for c in range(NCORE) for s in range(BPC)]
    return cores, budgets, perm


def _emit(tc, nc, d, out_ap, budgets):
    ctx = ExitStack()
    with ctx:
        wp = ctx.enter_context(tc.tile_pool(name="weights", bufs=1))
        xp = ctx.enter_context(tc.tile_pool(name="x", bufs=2))
        vp = ctx.enter_context(tc.tile_pool(name="v", bufs=2))
        qkp = ctx.enter_context(tc.tile_pool(name="qk", bufs=2))
        epool = ctx.enter_context(tc.tile_pool(name="exp", bufs=8))
        sp = ctx.enter_context(tc.tile_pool(name="small", bufs=8))
        tp = ctx.enter_context(tc.tile_pool(name="tail", bufs=1))
        ps_big = ctx.enter_context(tc.tile_pool(name="psA", bufs=2, space="PSUM"))
        ps_sc = ctx.enter_context(tc.tile_pool(name="psS", bufs=3, space="PSUM"))
        ps_sm = ctx.enter_context(tc.tile_pool(name="psW", bufs=3, space="PSUM"))

        def load(name, shape, dt=F32):
            t = wp.tile(list(shape), dt, tag=name)
            nc.sync.dma_start(t[:], d[name])
            return t

        # DMA emission in first-use order: sample-0 activations and the QKV
        # weights first, tail-only weights after the sample loop so they
        # stream in while attention computes.
        xts, xqs, mks = {}, {}, {}

        def prefetch(s):
            if s >= BPC or s in xts:
                return
            xT_ = xp.tile([128, KC, S], F16, tag="xT", name=f"xt{s}")
            nc.sync.dma_start(xT_[:], d["xT"][s])
            xts[s] = xT_
            xqs[s] = load(f"xq{s}", (128, KC, budgets[s] * 128), F16)
            mks[s] = load(f"mk{s}", (128, budgets[s], 2))

        prefetch(0)
        wv = load("wvT", (128, KC, H), F16)
        wq = load("wqT", (128, KC, NH * 128), F16)
        wk = load("wkT", (128, KC, NH * 128), F16)
        bq = load("bq", (HD, NH))
        bk = load("bk", (HD, NH))
        bv = load("bv", (128, H))

        ones = wp.tile([128, 1], F16, tag="ones")
        nc.vector.memset(ones[:], 1.0)

        c_pad = tp.tile([128, NH, 2 * BPC], F16, tag="c_pad")
        nc.vector.memset(c_pad[:], 0.0)
        combT = tp.tile([128, JC, BPC], F16, tag="combT")

        def mm(ps_ap, lhsT, rhs, start, stop):
            nc.tensor.matmul(ps_ap, lhsT=lhsT, rhs=rhs, start=start, stop=stop)

        for smp in range(BPC):
            prefetch(smp + 1)  # next sample's loads stream under this compute
            QB = budgets[smp]
            LQ = QB * 128
            xT = xts[smp]

            # V projection: v[s, o] with sequence position on partitions
            v_sb = vp.tile([128, QC, H], F16, tag="v")
            for sc in range(QC):
                for half in range(2):
                    pv_full = ps_big.tile([128, 512], F32, tag="a", name="pv")
                    pv = pv_full[:, :384]
                    for kc in range(KC):
                        mm(pv[:], xT[:, kc, sc * 128:(sc + 1) * 128],
                           wv[:, kc, half * 384:(half + 1) * 384], kc == 0, kc == KC - 1)
                    nc.vector.tensor_tensor(v_sb[:, sc, half * 384:(half + 1) * 384],
                                            pv[:], bv[:, half * 384:(half + 1) * 384], ALU.add)

            # Q over gathered span queries; K over the full sequence
            xq = xqs[smp]
            q_sb = qkp.tile([HD, NH, LQ], F16, tag="q")
            k_sb = qkp.tile([HD, NH, S], F16, tag="k")
            for h in range(NH):
                pq = ps_big.tile([128, S], F32, tag="a", name="pq")
                for kc in range(KC):
                    mm(pq[:, :LQ], wq[:, kc, h * 128:(h + 1) * 128], xq[:, kc, :],
                       kc == 0, kc == KC - 1)
                nc.vector.tensor_scalar_add(q_sb[:, h, :], pq[:HD, :LQ], bq[:, h:h + 1])
                pk = ps_big.tile([128, S], F32, tag="a", name="pk")
                for kc in range(KC):
                    mm(pk[:], wk[:, kc, h * 128:(h + 1) * 128], xT[:, kc, :],
                       kc == 0, kc == KC - 1)
                nc.vector.tensor_scalar_add(k_sb[:, h, :], pk[:HD, :], bk[:, h:h + 1])

            # attention, phase-split across heads so the scheduler can pack PE:
            # phase 1: all scores+exp+meff; phase 2: all pw/pc
            exps, meffs = [], []
            for h in range(NH):
                exp_sb = epool.tile([128, QB, S], F16, tag="exp")
                sums = sp.tile([128, QB, 1], F32, tag="sums")
                for qc in range(QB):
                    ps = ps_sc.tile([128, S], F32, tag="ps")
                    mm(ps[:], q_sb[:, h, qc * 128:(qc + 1) * 128], k_sb[:, h, :],
                       True, True)
                    nc.scalar.activation(exp_sb[:, qc, :], ps[:], AF.Exp, scale=SCALE,
                                         accum_out=sums[:, qc, :])
                rec = sp.tile([128, QB, 1], F32, tag="rec")
                nc.vector.reciprocal(rec[:], sums[:])
                meff = sp.tile([128, QB, 2], F16, tag="meff")
                for qc in range(QB):
                    nc.vector.tensor_scalar_mul(meff[:, qc, :], mks[smp][:, qc, :],
                                                rec[:, qc, :])
                exps.append(exp_sb)
                meffs.append(meff)
            for h in range(NH):
                exp_sb, meff = exps[h], meffs[h]
                w_sb = sp.tile([128, QC, 2], F16, tag="wsb")
                for ks in range(QC):
                    pw = ps_sm.tile([128, 2], F32, tag="sm")
                    for qc in range(QB):
                        mm(pw[:], exp_sb[:, qc, ks * 128:(ks + 1) * 128], meff[:, qc, :],
                           qc == 0, qc == QB - 1)
                    nc.vector.tensor_copy(w_sb[:, ks, :], pw[:])
                pc = ps_sm.tile([HD, 2], F32, tag="sm")
                for kc in range(QC):
                    mm(pc[:], v_sb[:, kc, h * HD:(h + 1) * HD], w_sb[:, kc, :],
                       kc == 0, kc == QC - 1)
                nc.vector.tensor_copy(c_pad[:HD, h, smp:smp + 1], pc[:, 0:1])
                nc.vector.tensor_copy(c_pad[:HD, h, BPC + smp:BPC + smp + 1], pc[:, 1:2])

        # tail-only constants (DMAs overlap with the attention phase above)
        wo = load("woT", (128, NH, H), F16)
        wco = load("wcoT", (128, NH, H), F16)
        atx = load("atx", (128, JC, NL + 1), F16)
        bo = load("bo", (128, KC))
        bco = load("bco", (128, KC))
        s1r = load("s1r", (BPC, NL))
        c0r = load("c0r", (BPC, NL))
        nc.sync.dma_start(combT[:, 4 * KC:5 * KC, :], d["posT"])

        # tail: e = c @ Wo.T + bo (and cross via folded Wco), concat blocks
        for c in range(KC):
            pe_ = ps_sm.tile([128, 2 * BPC], F32, tag="sm")
            for h in range(NH):
                mm(pe_[:], wo[:, h, c * 128:(c + 1) * 128], c_pad[:, h, :], h == 0, h == NH - 1)
            nc.scalar.activation(combT[:, c, :], pe_[:, 0:BPC], AF.Identity, bias=bo[:, c:c + 1])
            nc.scalar.activation(combT[:, KC + c, :], pe_[:, BPC:2 * BPC], AF.Identity,
                                 bias=bo[:, c:c + 1])
            px = ps_sm.tile([128, BPC], F32, tag="sm")
            for h in range(NH):
                mm(px[:], wco[:, h, c * 128:(c + 1) * 128], c_pad[:, h, BPC:2 * BPC],
                   h == 0, h == NH - 1)
            nc.scalar.activation(combT[:, 5 * KC + c, :], px[:], AF.Identity, bias=bco[:, c:c + 1])
            nc.vector.tensor_tensor(combT[:, 2 * KC + c, :], combT[:, c, :],
                                    combT[:, KC + c, :], ALU.mult)
            nc.vector.tensor_tensor(combT[:, 3 * KC + c, :], combT[:, c, :],
                                    combT[:, KC + c, :], ALU.subtract)

        # folded LayerNorm + classifier
        sq = tp.tile([128, JC, BPC], F16, tag="sq")
        nc.scalar.square(sq[:], combT[:])
        pxa = ps_sm.tile([BPC, NL + 1], F32, tag="sm")
        psq = ps_sm.tile([BPC, 1], F32, tag="sm")
        for j in range(JC):
            mm(pxa[:], combT[:, j, :], atx[:, j, :], j == 0, j == JC - 1)
            mm(psq[:], sq[:, j, :], ones[:], j == 0, j == JC - 1)
        mu = tp.tile([BPC, 1], F32, tag="mu")
        nc.vector.tensor_scalar_mul(mu[:], pxa[:, NL:NL + 1], 1.0 / SIX_H)
        msq = tp.tile([BPC, 1], F32, tag="msq")
        nc.vector.tensor_scalar_mul(msq[:], psq[:], 1.0 / SIX_H)
        mu2 = tp.tile([BPC, 1], F32, tag="mu2")
        nc.scalar.square(mu2[:], mu[:])
        var = tp.tile([BPC, 1], F32, tag="var")
        nc.vector.tensor_tensor(var[:], msq[:], mu2[:], ALU.subtract)
        epsb = tp.tile([BPC, 1], F32, tag="epsb")
        nc.vector.memset(epsb[:], LN_EPS)
        sd = tp.tile([BPC, 1], F32, tag="sd")
        nc.scalar.activation(sd[:], var[:], AF.Sqrt, bias=epsb[:])
        rs = tp.tile([BPC, 1], F32, tag="rs")
        nc.vector.reciprocal(rs[:], sd[:])
        t1 = tp.tile([BPC, NL], F32, tag="t1")
        nc.vector.tensor_scalar_mul(t1[:], s1r[:], mu[:])
        t2 = tp.tile([BPC, NL], F32, tag="t2")
        nc.vector.tensor_tensor(t2[:], pxa[:, 0:NL], t1[:], ALU.subtract)
        t3 = tp.tile([BPC, NL], F32, tag="t3")
        nc.vector.tensor_scalar_mul(t3[:], t2[:], rs[:])
        lg = tp.tile([BPC, NL], F32, tag="lg")
        nc.vector.tensor_tensor(lg[:], t3[:], c0r[:], ALU.add)
        nc.sync.dma_start(out_ap, lg[:])


_CACHED = {}


def _build(budgets):
    if budgets in _CACHED:
        return _CACHED[budgets]
    nc = bacc.Bacc(trn_type="TRN2", debug=False, num_devices=NCORE)
    d = {}

    def din(name, shape, dt=F32):
        d[name] = nc.dram_tensor(name, list(shape), dt, kind="ExternalInput").ap()

    din("xT", (BPC, 128, KC, S), F16)
    din("wqT", (128, KC, NH * 128), F16)
    din("wkT", (128, KC, NH * 128), F16)
    din("wvT", (128, KC, H), F16)
    din("woT", (128, NH, H), F16)
    din("wcoT", (128, NH, H), F16)
    din("bq", (HD, NH))
    din("bk", (HD, NH))
    din("bv", (128, H))
    din("bo", (128, KC))
    din("bco", (128, KC))
    din("atx", (128, JC, NL + 1), F16)
    din("s1r", (BPC, NL))
    din("c0r", (BPC, NL))
    for s in range(BPC):
        din(f"mk{s}", (128, budgets[s], 2))
        din(f"xq{s}", (128, KC, budgets[s] * 128), F16)
    din("posT", (128, KC, BPC), F16)
    out_ap = nc.dram_tensor("out", [BPC, NL], F32, kind="ExternalOutput").ap()

    with tile.TileContext(nc) as tc:
        _emit(tc, nc, d, out_ap, budgets)
    nc.compile()
    _CACHED[budgets] = nc
    return nc


def run(inputs, trace=False):
    shared = _prep_shared(inputs)
    cores, budgets, perm = _prep_percore(inputs)
    nc = _build(budgets)
    in_maps = [dict(shared, **cores[c]) for c in range(NCORE)]
    res = run_bass_kernel_spmd(nc, in_maps, core_ids=list(range(NCORE)), trace=trace)
    stacked = np.concatenate([res.results[c]["out"] for c in range(NCORE)], axis=0)
    out = np.empty_like(stacked)
    out[perm] = stacked
    return np.ascontiguousarray(out, dtype=np.float32), res


def kernel(**inputs):
    out, _ = run(inputs, trace=False)
    return out
